# revision 16
# baseline (speedup 1.0000x reference)
"""DeepEMD Trainium2 kernel: batched 49x49 entropic-OT (Sinkhorn) similarity.

Strategy (8 NeuronCores, data-parallel over batch):
- Each core gets 128 batches. Host prepacks, per (chunk j of 128 channels,
  batch b), an augmented matrix A = [Q | P | 1] (128 x 99) in bf16 hi + bf16 lo
  (lossless-ish fp32 split), laid out so DMA loads are large contiguous runs.
- PE computes the Gram G_b = A^T A (99x99, fp32 PSUM) with 12 accumulating
  bf16 matmuls per batch: hi.hi + hi.lo + lo.hi (lo.lo term ~1e-5 negligible).
  G_b contains Q^T P, P^T Q, column sums (ones row) and diag blocks -> every
  downstream quantity (similarity map, norms, weight vectors) is a cheap
  fixup of G.
- A per-batch SBUF->SBUF DMA flattens G_b into row b of a [128, 99*99] tile:
  everything after that runs batch-on-partitions, full 128-lane DVE.
- Sinkhorn runs in the *linear* domain (K = exp((sim-1)/eps)) with
  Gauss-Seidel updates us = r/(K vs), vs = c/(K^T us). The reference's 100
  log-domain iterations converge to ~1e-12 by 20 iters on this data; ITERS
  linear f32 iterations reach the f32 floor (~1e-5 at 10).
- logits[b] = T * sum(flow * sim) = T * us^T (K.sim) vs, computed with one
  broadcast-mul + reduce + small ops.
"""

import os
import sys

import numpy as np

sys.path.insert(0, "/opt/trn_rl_repo")

import concourse.bass as bass
import concourse.bacc as bacc
import concourse.mybir as mybir
from concourse import tile
from concourse.bass_utils import run_bass_kernel_spmd

import ml_dtypes

B_FULL, C, HW = 1024, 512, 49
NCORE = 8
BS = B_FULL // NCORE  # 128 batches per core
NCH = C // 128  # 4 chunks of 128 channels (PE contraction dim)
AC = 2 * HW + 1  # 99 augmented columns [Q | P | 1]
GRP = 16  # batches per DMA group
NGRP = BS // GRP
ITERS = 8
EPS_S = 0.05
TEMP = 12.5 / HW
EXP_BIAS = -4.0  # exp((sim-1)/eps) * e^16 rescale; cancels in us*K*vs

f32 = mybir.dt.float32
bf16 = mybir.dt.bfloat16
Alu = mybir.AluOpType
Act = mybir.ActivationFunctionType
AxX = mybir.AxisListType.X


def build_nc(debug=False):
    nc = bacc.Bacc(None, target_bir_lowering=False, debug=debug)
    hi = nc.declare_dram_parameter("aughi", [NCH, 128, BS, AC], bf16, isOutput=False)
    lo = nc.declare_dram_parameter("auglo", [NCH, 128, BS, AC], bf16, isOutput=False)
    outp = nc.declare_dram_parameter("out", [BS, 1], f32, isOutput=True)

    with tile.TileContext(nc) as tc:
        with (
            tc.tile_pool(name="big", bufs=1) as big,
            tc.tile_pool(name="stage", bufs=2) as stg,
            tc.tile_pool(name="gcopy", bufs=8) as gcp,
            tc.tile_pool(name="work", bufs=3) as wrk,
            tc.tile_pool(name="small", bufs=1) as sml,
            tc.tile_pool(name="psum", bufs=8, space="PSUM") as pp,
        ):
            flatG = big.tile([BS, AC * AC], f32, tag="flatG", name="flatG")

            # ---------------- Phase 1: DMA in + Gram + flatten ----------------
            JW = GRP * AC  # 1584 cols per chunk-slab
            for g in range(NGRP):
                th = stg.tile([128, NCH * JW], bf16, tag="h", name="hg")
                tl = stg.tile([128, NCH * JW], bf16, tag="l", name="lg")
                # one DMA per tensor per group: [c, j, b, col] iteration on
                # both sides, 3168B contiguous runs
                for src, dst in ((hi, th), (lo, tl)):
                    nc.sync.dma_start(
                        dst[:].rearrange("c (j w) -> c j w", w=JW),
                        src[:, :, g * GRP : (g + 1) * GRP, :].rearrange(
                            "j c b a -> c j (b a)"
                        ),
                    )
                for bb in range(GRP):
                    b = g * GRP + bb
                    ps = pp.tile([128, AC], f32, tag="gram", name="gram")
                    n_mm = 3 * NCH
                    k = 0
                    # weights widened to 128 cols to trigger fast-weight-load;
                    # the extra 29 rows of G are junk. The last batch of a
                    # slab keeps the plain 99-col weight (window would run
                    # off the slab end).
                    wid = AC if bb == GRP - 1 else 128
                    for j in range(NCH):
                        base = j * JW + bb * AC
                        hT = th[:, base : base + wid]
                        lT = tl[:, base : base + wid]
                        hR = th[:, base : base + AC]
                        lR = tl[:, base : base + AC]
                        for lhsT, rhs in ((hT, hR), (hT, lR), (lT, hR)):
                            nc.tensor.matmul(
                                ps[0:wid, :],
                                lhsT,
                                rhs,
                                start=(k == 0),
                                stop=(k == n_mm - 1),
                            )
                            k += 1
                    gs = gcp.tile([AC, AC], f32, tag="gs", name="gs")
                    nc.scalar.copy(gs[:], ps[0:AC, :])
                    # flatten [99, 99] -> one batch-major row
                    nc.scalar.dma_start(flatG[b : b + 1, :], gs[:])

            # ---------------- Phase 1.5: fixup to sim/K/marginals -------------
            G3 = flatG[:].rearrange("p (q c) -> p q c", c=AC)
            qtp = G3[:, 0:HW, HW : 2 * HW]  # [128, 49, 49] raw Q^T P
            ptq = G3[:, HW : 2 * HW, 0:HW]
            sq = flatG[:, (AC - 1) * AC : (AC - 1) * AC + HW]  # 1^T Q
            sp = flatG[:, (AC - 1) * AC + HW : (AC - 1) * AC + 2 * HW]  # 1^T P
            dq = flatG[:, 0 : HW * (AC + 1)].rearrange(
                "p (a b) -> p a b", b=AC + 1
            )[:, :, 0:1]  # diag(QtQ) [128, 49, 1]
            dp = flatG[:, HW * (AC + 1) : 2 * HW * (AC + 1)].rearrange(
                "p (a b) -> p a b", b=AC + 1
            )[:, :, 0:1]  # diag(PtP)

            def s49(tag):
                return sml.tile([BS, HW], f32, tag=tag, name=tag)

            inq, inp_, t1, t2 = s49("inq"), s49("inp"), s49("t1"), s49("t2")
            aq, ap_ = s49("aq"), s49("ap")
            w1, w2, us, vs = s49("w1"), s49("w2"), s49("us"), s49("vs")
            kv, rkv = s49("kv"), s49("rkv")
            s2 = sml.tile([BS, 1], f32, tag="s2", name="s2")
            ebias = sml.tile([BS, 1], f32, tag="ebias", name="ebias")
            nc.vector.memset(ebias[:], EXP_BIAS)
            lg = sml.tile([BS, 1], f32, tag="lg", name="lg")
            lgf = sml.tile([BS, 1], f32, tag="lgf", name="lgf")

            d3q = dq.squeeze(2)  # [128, 49] views
            d3p = dp.squeeze(2)

            for (sx, dx, inv) in ((sq, d3q, inq), (sp, d3p, inp_)):
                # u = d - s^2/C ; inv = rsqrt(u) via sqrt LUT + recip + Newton
                nc.vector.tensor_mul(t1[:], sx, sx)
                nc.vector.scalar_tensor_tensor(
                    t2[:], t1[:], -1.0 / C, dx, Alu.mult, Alu.add
                )
                nc.scalar.activation(t1[:], t2[:], Act.Sqrt)
                nc.vector.reciprocal(inv[:], t1[:])
                # Newton for rsqrt: y = y*(1.5 - 0.5*u*y^2)
                nc.vector.tensor_mul(t1[:], inv[:], inv[:])
                nc.vector.tensor_mul(t1[:], t1[:], t2[:])
                nc.vector.tensor_scalar(t1[:], t1[:], -0.5, 1.5, Alu.mult, Alu.add)
                nc.vector.tensor_mul(inv[:], inv[:], t1[:])

            rC = 1.0 / np.sqrt(float(C))
            nc.vector.scalar_tensor_tensor(aq[:], sq, rC, inq[:], Alu.mult, Alu.mult)
            nc.vector.scalar_tensor_tensor(ap_[:], sp, rC, inp_[:], Alu.mult, Alu.mult)

            simb = big.tile([BS, HW * HW], f32, tag="sim", name="sim")
            Kb = big.tile([BS, HW * HW], f32, tag="K", name="K")
            Ktb = big.tile([BS, HW * HW], f32, tag="Kt", name="Kt")
            KSb = big.tile([BS, HW * HW], f32, tag="KSim", name="KSim")
            b1 = wrk.tile([BS, HW * HW], f32, tag="w", name="b1")
            b3 = wrk.tile([BS, HW * HW], f32, tag="w", name="b3")
            simTb = wrk.tile([BS, HW * HW], f32, tag="w", name="simTb")

            def v3(t):  # [128, 49, 49] view of a [128, 2401] tile
                return t[:].rearrange("p (q c) -> p q c", c=HW)

            def v3t(t):  # transposed view (strides 1, 49)
                return t[:].rearrange("p (q c) -> p c q", c=HW)

            bq = inq[:].unsqueeze(2).broadcast_to([BS, HW, HW])
            bp = inp_[:].unsqueeze(1).broadcast_to([BS, HW, HW])
            nc.vector.tensor_mul(v3(b1), bq, bp)  # B1 = inq x inp
            nc.vector.tensor_mul(v3(simb), qtp, v3(b1))  # B2
            baq = aq[:].unsqueeze(2).broadcast_to([BS, HW, HW])
            bap = ap_[:].unsqueeze(1).broadcast_to([BS, HW, HW])
            nc.vector.tensor_mul(v3(b3), baq, bap)  # B3 = aq x ap
            nc.vector.scalar_tensor_tensor(
                v3(simb), v3(b3), -1.0, v3(simb), Alu.mult, Alu.add
            )  # sim = B2 - B3
            # transposed side via transposed views of B1/B3
            nc.vector.tensor_mul(v3(simTb), ptq, v3t(b1))
            nc.vector.scalar_tensor_tensor(
                v3(simTb), v3t(b3), -1.0, v3(simTb), Alu.mult, Alu.add
            )
            nc.scalar.activation(Kb[:], simb[:], Act.Exp, scale=1.0 / EPS_S,
                                 bias=ebias[:])
            nc.scalar.activation(Ktb[:], simTb[:], Act.Exp, scale=1.0 / EPS_S,
                                 bias=ebias[:])
            nc.vector.tensor_mul(KSb[:], Kb[:], simb[:])

            # weight vectors: w = relu(rowsum/49) + 0.001 (unnormalized; the
            # r-normalization cancels in the logits, the c-normalization is a
            # final 1/s2 scale)
            nc.vector.tensor_reduce(w1[:], qtp, axis=AxX, op=Alu.add)
            nc.vector.tensor_reduce(w2[:], ptq, axis=AxX, op=Alu.add)
            for w in (w1, w2):
                nc.vector.tensor_scalar(w[:], w[:], 1.0 / HW, 0.0, Alu.mult, Alu.max)
                nc.vector.tensor_scalar(w[:], w[:], 0.001, None, Alu.add)
            nc.vector.tensor_reduce(s2[:], w2[:], axis=AxX, op=Alu.add)

            # ---------------- Phase 2: Sinkhorn (Gauss-Seidel, linear) --------
            tb = wrk.tile([BS, HW * HW], f32, tag="w", name="tb")
            bvs = vs[:].unsqueeze(1).broadcast_to([BS, HW, HW])
            bus = us[:].unsqueeze(1).broadcast_to([BS, HW, HW])
            for it in range(ITERS):
                if it == 0:
                    nc.vector.tensor_reduce(kv[:], v3(Kb), axis=AxX, op=Alu.add)
                else:
                    nc.vector.tensor_mul(v3(tb), v3(Kb), bvs)
                    nc.vector.tensor_reduce(kv[:], v3(tb), axis=AxX, op=Alu.add)
                nc.vector.reciprocal(rkv[:], kv[:])
                nc.vector.tensor_mul(us[:], w1[:], rkv[:])
                nc.vector.tensor_mul(v3(tb), v3(Ktb), bus)
                nc.vector.tensor_reduce(kv[:], v3(tb), axis=AxX, op=Alu.add)
                nc.vector.reciprocal(rkv[:], kv[:])
                nc.vector.tensor_mul(vs[:], w2[:], rkv[:])

            # ---------------- Phase 3: logits ---------------------------------
            nc.vector.tensor_mul(v3(tb), v3(KSb), bvs)
            nc.vector.tensor_reduce(kv[:], v3(tb), axis=AxX, op=Alu.add)
            nc.vector.tensor_mul(kv[:], kv[:], us[:])
            nc.vector.tensor_reduce(lg[:], kv[:], axis=AxX, op=Alu.add)
            nc.vector.reciprocal(rkv[:, 0:1], s2[:])
            nc.vector.scalar_tensor_tensor(
                lgf[:], lg[:], TEMP, rkv[:, 0:1], Alu.mult, Alu.mult
            )  # (lg * T) / s2
            nc.sync.dma_start(outp[:, :], lgf[:])

    nc.compile()
    return nc


_NC = None


def _get_nc():
    global _NC
    if _NC is None:
        _NC = build_nc()
    return _NC


def _prep_in_maps(feature_map1, feature_map2):
    q = np.ascontiguousarray(np.asarray(feature_map1, dtype=np.float32)).reshape(
        B_FULL, C, HW
    )
    p = np.ascontiguousarray(np.asarray(feature_map2, dtype=np.float32)).reshape(
        B_FULL, C, HW
    )
    in_maps = []
    for i in range(NCORE):
        sl = slice(i * BS, (i + 1) * BS)
        aug = np.empty((NCH, 128, BS, AC), np.float32)
        aug[..., AC - 1] = 1.0
        aug[..., 0:HW] = q[sl].reshape(BS, NCH, 128, HW).transpose(1, 2, 0, 3)
        aug[..., HW : 2 * HW] = p[sl].reshape(BS, NCH, 128, HW).transpose(1, 2, 0, 3)
        hi = aug.astype(ml_dtypes.bfloat16)
        lo = (aug - hi.astype(np.float32)).astype(ml_dtypes.bfloat16)
        in_maps.append({"aughi": hi, "auglo": lo})
    return in_maps


def run(feature_map1, feature_map2, trace=False):
    in_maps = _prep_in_maps(feature_map1, feature_map2)
    nc = _get_nc()
    res = run_bass_kernel_spmd(nc, in_maps, core_ids=list(range(NCORE)), trace=trace)
    out = np.concatenate(
        [np.asarray(res.results[i]["out"]).reshape(BS) for i in range(NCORE)]
    ).astype(np.float32)
    return out, res


def kernel(feature_map1, feature_map2):
    out, _ = run(feature_map1, feature_map2, trace=False)
    return out
